# revision 47
# baseline (speedup 1.0000x reference)
"""Multi-head attention (B=2, L=2048, H=1024, NH=16) on 8 TRN2 NeuronCores.

Sharding: data-parallel over batch (2) x tensor-parallel over heads (4 groups
of 4 heads).  core = b*4 + g handles batch b, heads [4g, 4g+4).  Wq/Wk/Wv are
split column-wise, Wo row-wise; each core produces a partial [L, H] output
that the host sums per batch (the row-parallel all-reduce done host-side).

Device math (per core), all matmuls bf16 inputs / fp32 PSUM accumulation:
  QT = (Wq*0.125)^T x^T          [256, 2048]  (softmax scale folded into Wq)
  KT = Wk^T y^T                  [256, 2048]
  V  = y Wv                      [2048, 256] stored as V_aug [lk, 4*(64+1)]
                                 with a ones column per head
  per head h, per 1024-wide lq chunk:
    S^T[lk, lq] = KT_h^T QT_h    (contraction d=64)
    P^T = exp(S^T)               ScalarE, PSUM -> SBUF bf16 (no max-subtract:
                                 logits are O(1) by construction)
    O^T_aug[65, lq] = sum_lk V_aug_h^T P^T   (row 64 = softmax denominators)
    O'^T = O^T * broadcast(1/sums)           DVE recip + GpSimd partition bcast
  out[lq, 1024] += O'^T_cat^T Wo  (partial; host sums the 4 head-groups)

The emission order keeps one continuous exp stream on ScalarE (the pacing
engine) from ~28us in: the V projection, the remaining Q/K projection
groups, and stage 3 of chunk 0 ride inside the stream as per-lk-tile hook
work; inputs are host-packed partition-major so every DMA is 128 contiguous
runs (SP descriptor generation otherwise dominates startup).
"""

import numpy as np
import ml_dtypes

B, L, H, NH, D = 2, 2048, 1024, 16, 64
GP = 4            # head-groups (tensor-parallel factor)
CH = H // GP      # 256 local projection cols per core
HL = NH // GP     # 4 local heads
LQ = 1024         # lq chunk size
NLQ = L // LQ
NKT = L // 128    # 16 lk tiles
BF16 = ml_dtypes.bfloat16

_CACHE = {}


def _build():
    import concourse.mybir as mybir
    import concourse.tile as tile
    from concourse import bacc

    dt = mybir.dt
    f32, bf16 = dt.float32, dt.bfloat16
    Exp = mybir.ActivationFunctionType.Exp

    nc = bacc.Bacc("TRN2", target_bir_lowering=False, debug=False, use_seq_codegen=True)
    # all inputs host-packed partition-major so each DMA is 128 long
    # contiguous runs (SP descriptor generation is the startup bottleneck)
    xT = nc.declare_dram_parameter("xT", [128, NLQ, 2, 8, 512], bf16,
                                   isOutput=False)
    yT = nc.declare_dram_parameter("yT", [128, NLQ, 2, 8, 512], bf16,
                                   isOutput=False)
    wq = nc.declare_dram_parameter("wq", [128, 2, 8, 128], bf16,
                                   isOutput=False)
    wk = nc.declare_dram_parameter("wk", [128, 2, 8, 128], bf16,
                                   isOutput=False)
    wv = nc.declare_dram_parameter("wv", [128, 8, CH], bf16, isOutput=False)
    wo = nc.declare_dram_parameter("wo", [128, 2, H], bf16, isOutput=False)
    out = nc.declare_dram_parameter("out", [L, H], bf16, isOutput=True)

    with tile.TileContext(nc) as tc:
        with (
            tc.tile_pool(name="w", bufs=1) as wpool,
            tc.tile_pool(name="acts", bufs=1) as apool,
            tc.tile_pool(name="psA", bufs=3, space="PSUM") as psA,
            tc.tile_pool(name="psO", bufs=1, space="PSUM") as psO,
            tc.tile_pool(name="pt", bufs=8) as ptpool,
            tc.tile_pool(name="oT", bufs=2) as otpool,
            tc.tile_pool(name="sm", bufs=2) as smpool,
            tc.tile_pool(name="osb", bufs=6) as opool,
        ):
            # prefetch the exp activation table while input DMAs run
            dummy = smpool.tile([1, 8], f32, tag="dummy")
            nc.vector.memset(dummy, 0.0)
            nc.scalar.activation(dummy, dummy, Exp)

            # PE warm-up: stream zero matmuls through the array while the
            # input DMAs run, so the tensor engine's DVFS ramp (0.65 ->
            # 2.4 GHz over ~3us of continuous work) completes before the
            # first real projection -- and ends just as its inputs land
            wz = wpool.tile([128, 128], bf16, tag="wz")
            az = wpool.tile([128, 512], bf16, tag="az")
            nc.vector.memset(wz, 0.0)
            nc.vector.memset(az, 0.0)
            psW = psO.tile([128, 512], f32, tag="psO", name="psW")
            for i in range(14):
                nc.tensor.matmul(psW, lhsT=wz, rhs=az,
                                 start=(i == 0), stop=(i == 13))

            # ---- input DMAs, ordered so the PE can run projection work
            # continuously through the DMA-bound startup window: y chunk 0
            # + K weights first (K ct0 + V groups), then x chunk 0 + Q
            # weights (Q ct0 lh0 -> first exp), then the rest ------------
            wk_sb = wpool.tile([128, 2, 8, 128], bf16, tag="wk")
            wq_sb = wpool.tile([128, 2, 8, 128], bf16, tag="wq")
            yT_sb = apool.tile([128, NLQ, 2, 8, 512], bf16, tag="yT")
            xT_sb = apool.tile([128, NLQ, 2, 8, 512], bf16, tag="xT")
            wv_sb = wpool.tile([128, 8, CH], bf16, tag="wv")
            for hp in range(4):
                nc.sync.dma_start(yT_sb[:, 0, 0, 2 * hp:2 * hp + 2],
                                  yT[:, 0, 0, 2 * hp:2 * hp + 2])
                if hp == 0:
                    nc.sync.dma_start(wk_sb[:, 0, 0:4], wk[:, 0, 0:4])
                elif hp == 1:
                    nc.sync.dma_start(wk_sb[:, 0, 4:8], wk[:, 0, 4:8])
            nc.sync.dma_start(wv_sb, wv[:, :, :])
            nc.sync.dma_start(yT_sb[:, 0, 1], yT[:, 0, 1])
            nc.sync.dma_start(xT_sb[:, 0, 0], xT[:, 0, 0])
            nc.sync.dma_start(wq_sb[:, 0], wq[:, 0])
            nc.sync.dma_start(xT_sb[:, 0, 1], xT[:, 0, 1])
            for sl in range(2):
                nc.sync.dma_start(yT_sb[:, 1, sl], yT[:, 1, sl])
            nc.sync.dma_start(wk_sb[:, 1], wk[:, 1])
            nc.sync.dma_start(wq_sb[:, 1], wq[:, 1])
            for sl in range(2):
                nc.sync.dma_start(xT_sb[:, 1, sl], xT[:, 1, sl])
            wo_sb = wpool.tile([128, 2, H], bf16, tag="wo")
            nc.sync.dma_start(wo_sb, wo[:, :, :])

            qT_sb = apool.tile([128, 2, L], bf16, tag="qT")
            kT_sb = apool.tile([128, 2, L], bf16, tag="kT")
            vaug_sb = apool.tile([128, NKT, HL * 65], bf16, tag="vaug")

            def proj_group(w_sb, act_sb, dst, ct, lh, sl):
                # dst[:, ct, lh*LQ+sl*512 : +512] via one 8-matmul psum group
                ps = psA.tile([128, LQ], f32, tag="psA")
                off = lh * LQ + sl * 512
                for ht in range(8):
                    nc.tensor.matmul(
                        ps[:, 0:512],
                        lhsT=w_sb[:, ct, ht, :],
                        rhs=act_sb[:, lh, sl, ht, :],
                        start=(ht == 0), stop=(ht == 7),
                    )
                nc.vector.tensor_copy(dst[:, ct, off:off + 512], ps[:, 0:512])

            def v_group(lkt):
                # one lk tile of V_aug[lk, 4*(64+1)] bf16 (+ones col per head)
                psv = psA.tile([128, LQ], f32, tag="psA")
                for ht in range(8):
                    nc.tensor.matmul(
                        psv[:, :CH],
                        lhsT=yT_sb[:, lkt // 8, (lkt % 8) // 4, ht,
                                   (lkt % 4) * 128:(lkt % 4 + 1) * 128],
                        rhs=wv_sb[:, ht, :],
                        start=(ht == 0), stop=(ht == 7),
                    )
                vh = vaug_sb[:, lkt, :].rearrange("p (h e) -> p h e", h=HL)
                nc.vector.tensor_copy(
                    vh[:, :, 0:64],
                    psv[:, :CH].rearrange("p (h e) -> p h e", h=HL))
                nc.vector.memset(vh[:, :, 64], 1.0)

            def s3_piece(ci, oT_sb, mt, pool=None, act_copy=False):
                # both 512-wide output halves in one psum tile -> one copy
                # and one full-width contiguous store (amortizes the
                # slot->copy->store latency that paces the tail)
                pool = pool if pool is not None else psO
                pso = pool.tile([128, LQ], f32,
                                tag="psO" if pool is psO else "psA")
                for nt in range(2):
                    for kt in range(2):
                        nc.tensor.matmul(
                            pso[:, nt * 512:(nt + 1) * 512],
                            lhsT=oT_sb[:, kt, mt * 128:(mt + 1) * 128],
                            rhs=wo_sb[:, kt, nt * 512:(nt + 1) * 512],
                            start=(kt == 0), stop=(kt == 1),
                        )
                osb = opool.tile([128, LQ], bf16, tag="osb")
                if act_copy:
                    nc.scalar.copy(osb, pso)
                else:
                    nc.vector.tensor_copy(osb, pso)
                nc.sync.dma_start(
                    out[ci * LQ + mt * 128:ci * LQ + (mt + 1) * 128, :],
                    osb)

            def s3q(ci, oT_sb, mt, nt, act_copy=False):
                # one 512-wide quarter of a stage-3 piece as two ~213ns PE
                # thunks for consecutive hook slots: fine-grained filler
                # that keeps the PE streaming (and clocked up) through the
                # ACT-paced heads
                cell = {}

                def t0():
                    cell["ps"] = psA.tile(
                        [128, 512], f32, tag="psA",
                        name=f"s3q{ci}_{mt}_{nt}")
                    nc.tensor.matmul(
                        cell["ps"],
                        lhsT=oT_sb[:, 0, mt * 128:(mt + 1) * 128],
                        rhs=wo_sb[:, 0, nt * 512:(nt + 1) * 512],
                        start=True, stop=False,
                    )

                def t1():
                    ps = cell["ps"]
                    nc.tensor.matmul(
                        ps,
                        lhsT=oT_sb[:, 1, mt * 128:(mt + 1) * 128],
                        rhs=wo_sb[:, 1, nt * 512:(nt + 1) * 512],
                        start=False, stop=True,
                    )
                    osb = opool.tile([128, 512], bf16, tag="osb",
                                     name=f"osbq{ci}_{mt}_{nt}")
                    if act_copy:
                        nc.scalar.copy(osb, ps)
                    else:
                        nc.vector.tensor_copy(osb, ps)
                    nc.sync.dma_start(
                        out[ci * LQ + mt * 128:ci * LQ + (mt + 1) * 128,
                            nt * 512:(nt + 1) * 512],
                        osb)
                return t0, t1

            def emit_S(ci, h, lkt):
                po, ct2 = h % 2, h // 2
                psS = psA.tile([128, LQ], f32, tag="psA")
                for sl in range(LQ // 512):
                    nc.tensor.matmul(
                        psS[:, sl * 512:(sl + 1) * 512],
                        lhsT=kT_sb[64 * po:64 * po + 64, ct2,
                                   lkt * 128:(lkt + 1) * 128],
                        rhs=qT_sb[64 * po:64 * po + 64, ct2,
                                  ci * LQ + sl * 512:
                                  ci * LQ + (sl + 1) * 512],
                        start=True, stop=True,
                    )
                return psS

            pipe = {}

            def s2(ci, h, oT_sb, extra=None, nxt=None, nq=2):
                # per lk tile: S^T matmuls -> exp -> O^T accumulation.  S
                # matmuls run one lkt ahead of the O matmuls (and prefetch
                # across head boundaries via `nxt`) so exp(k+1)'s input is
                # ready the moment ScalarE finishes exp(k).
                po, ct2 = h % 2, h // 2
                psO_h = psO.tile([128, LQ], f32, tag="psO")
                psS = pipe.pop("psS", None)
                if psS is None:
                    psS = emit_S(ci, h, 0)
                for lkt in range(NKT):
                    pt = ptpool.tile([128, LQ], bf16, tag="pt")
                    nc.scalar.activation(pt, psS, Exp)
                    if lkt + 1 < NKT:
                        psS = emit_S(ci, h, lkt + 1)
                    elif nxt is not None:
                        pipe["psS"] = emit_S(nxt[0], nxt[1], 0)
                    if extra is not None:
                        extra(lkt)
                    for sl in range(LQ // 512):
                        nc.tensor.matmul(
                            psO_h[0:65, sl * 512:(sl + 1) * 512],
                            lhsT=vaug_sb[:, lkt, h * 65:(h + 1) * 65],
                            rhs=pt[:, sl * 512:(sl + 1) * 512],
                            start=(lkt == 0), stop=(lkt == NKT - 1),
                        )
                # DVE drains free the psO banks for the next head (sums
                # via tensor_copy: partition-remapping 64 -> 0); normalize
                # then runs from SBUF in nq lq pieces.  The last head uses
                # nq=4 with interleaved quarter-drains so the tail's first
                # stage-3 pieces unlock as soon as possible.
                sums = smpool.tile([1, LQ], f32, tag="sums")
                oc = smpool.tile([64, LQ], f32, tag="oc")
                w = LQ // nq
                nc.vector.tensor_copy(sums, psO_h[64:65, :])
                if nq == 2:
                    nc.vector.tensor_copy(oc, psO_h[0:64, :])
                for hs in range(nq):
                    c = slice(hs * w, (hs + 1) * w)
                    recip = smpool.tile([1, 512], f32, tag="recip")
                    nc.vector.reciprocal_approx_fast(recip[:, :w],
                                                     sums[:, c])
                    bcast = smpool.tile([64, 512], f32, tag="bcast")
                    nc.gpsimd.partition_broadcast(bcast[:, :w], recip[:, :w])
                    if nq != 2:
                        nc.vector.tensor_copy(oc[:, c], psO_h[0:64, c])
                    nc.vector.tensor_mul(
                        oT_sb[64 * po:64 * po + 64, ct2, c], oc[:, c],
                        bcast[:, :w])

            # ---- emission order: pack the DMA-bound startup window with
            # all the projection work chunk-0 y/x can feed (K ct0 lh0, V
            # lk 0:1024, Q ct0 lh0), then run a continuous per-lkt exp
            # stream; remaining projections and stage 3 ride inside the
            # stream as per-deadline hook work ----------------------------
            oT = [otpool.tile([128, 2, LQ], bf16, tag="oT", name=f"oT{i}")
                  for i in range(NLQ)]
            proj_group(wk_sb, yT_sb, kT_sb, 0, 0, 0)
            for j in range(4):
                v_group(j)
            proj_group(wk_sb, yT_sb, kT_sb, 0, 0, 1)
            for j in range(4, 8):
                v_group(j)
            proj_group(wq_sb, xT_sb, qT_sb, 0, 0, 0)
            proj_group(wq_sb, xT_sb, qT_sb, 0, 0, 1)

            def make_hook(sched):
                # sched: lkt -> list of thunks
                def hook(lkt):
                    for job in sched.get(lkt, ()):
                        job()
                return hook

            vj = [(lambda j=j: v_group(j)) for j in range(NKT)]
            pj = lambda w, a, d, ct, lh, sl: (  # noqa: E731
                lambda: proj_group(w, a, d, ct, lh, sl))

            def pj2(w_sb, act_sb, dst, ct, lh, sl):
                # one projection psum group split into two 4-matmul thunks
                cell = {}

                def half(r):
                    def thunk():
                        if r == 0:
                            cell["ps"] = psA.tile(
                                [128, LQ], f32, tag="psA",
                                name=f"pjps{ct}_{lh}_{sl}_{id(w_sb) % 97}")
                        ps = cell["ps"]
                        for ht in range(4 * r, 4 * r + 4):
                            nc.tensor.matmul(
                                ps[:, 0:512],
                                lhsT=w_sb[:, ct, ht, :],
                                rhs=act_sb[:, lh, sl, ht, :],
                                start=(ht == 0), stop=(ht == 7),
                            )
                        if r == 1:
                            nc.vector.tensor_copy(
                                dst[:, ct, lh * LQ + sl * 512:
                                    lh * LQ + (sl + 1) * 512], ps[:, 0:512])
                    return thunk
                return half(0), half(1)
            def sched(items):
                # items: (slot, thunk) pairs -> hook.  Hooks are whole
                # psum groups in a single slot: fewer polluted iterations
                # keeps the S/O weight-load pipeline clean.
                d = {}
                for s, t in items:
                    d.setdefault(s, []).append(t)
                return make_hook(d)

            def s3h(ci, mt, nt, act_copy=False):
                a, b = s3q(ci, oT[ci], mt, nt, act_copy)
                return lambda: (a(), b())

            # head 0 carries the second half of V (v_group(j) just before
            # the O-mms of lkt j) plus the K ct0 lh1 groups (whose lk
            # tiles are first consumed from lkt 8 on)
            h0_sched = {j - 1: [vj[j]] for j in range(8, NKT)}
            h0_sched[2] = [pj(wk_sb, yT_sb, kT_sb, 0, 1, 0)]
            h0_sched[5] = [pj(wk_sb, yT_sb, kT_sb, 0, 1, 1)]
            h0_hook = make_hook(h0_sched)
            # per-deadline balance: Q ct1 lh0 and K(1,0,0) are consumed
            # right after h1 (S prefetch of head 2); the later K ct1
            # pieces are consumed lk-tile by lk-tile through h2
            h1_hook = sched([
                (0, pj(wq_sb, xT_sb, qT_sb, 1, 0, 0)),
                (5, pj(wq_sb, xT_sb, qT_sb, 1, 0, 1)),
                (10, pj(wk_sb, yT_sb, kT_sb, 1, 0, 0))])
            h2_hook = sched([
                (0, pj(wk_sb, yT_sb, kT_sb, 1, 0, 1)),
                (5, pj(wk_sb, yT_sb, kT_sb, 1, 1, 0)),
                (9, pj(wk_sb, yT_sb, kT_sb, 1, 1, 1))])
            h3_hook = sched([
                (0, pj(wq_sb, xT_sb, qT_sb, 0, 1, 0)),
                (6, pj(wq_sb, xT_sb, qT_sb, 0, 1, 1))])
            h4_hook = sched([
                (0, pj(wq_sb, xT_sb, qT_sb, 1, 1, 0)),
                (6, pj(wq_sb, xT_sb, qT_sb, 1, 1, 1))])
            # chunk-0 stage 3 smears over the ACT-paced chunk-1 heads
            # (oT[0] is final after h3), one 512-wide half per slot
            h5_hook = sched([
                (1, s3h(0, 0, 0)), (4, s3h(0, 0, 1)), (7, s3h(0, 1, 0)),
                (10, s3h(0, 1, 1)), (13, s3h(0, 2, 0))])
            h6_hook = sched([
                (1, s3h(0, 2, 1)), (4, s3h(0, 3, 0)), (7, s3h(0, 3, 1)),
                (10, s3h(0, 4, 0)), (13, s3h(0, 4, 1))])
            h7_hook = sched([
                (0, s3h(0, 5, 0)), (4, s3h(0, 5, 1)), (8, s3h(0, 6, 0))])

            s2(0, 0, oT[0], extra=h0_hook, nxt=(0, 1))
            s2(0, 1, oT[0], extra=h1_hook, nxt=(0, 2))
            s2(0, 2, oT[0], extra=h2_hook, nxt=(0, 3))
            s2(0, 3, oT[0], extra=h3_hook, nxt=(1, 0))
            s2(1, 0, oT[1], extra=h4_hook, nxt=(1, 1))
            s2(1, 1, oT[1], extra=h5_hook, nxt=(1, 2))
            s2(1, 2, oT[1], extra=h6_hook, nxt=(1, 3))
            s2(1, 3, oT[1], extra=h7_hook, nq=4)
            # tail: the remaining chunk-0 stage-3 pieces fill the PE gap
            # while h7's first normalize quarter-chain runs, then chunk-1
            # stage 3 drains.  The first pieces copy out via ScalarE (the
            # DVE is still working through the normalize); later ones DVE.
            s3h(0, 6, 1, act_copy=True)()
            s3h(0, 7, 0, act_copy=True)()
            s3h(0, 7, 1, act_copy=True)()
            for mt in range(LQ // 128):
                s3_piece(1, oT[1], mt, pool=(psA if mt < 3 or mt % 2 else psO),
                         act_copy=(mt < 4))
    nc.compile()
    return nc


def _get_nc():
    if "nc" not in _CACHE:
        _CACHE["nc"] = _build()
    return _CACHE["nc"]


def _pack_pm(a, t):
    # [t*128, N] -> [128, t, N] partition-major
    return a.reshape(t, 128, -1).transpose(1, 0, 2)


def _pack_act(a):
    # x[b] [L, H] -> xT packed [128, NLQ(lh), 2(sl), 8(t), 512] bf16
    v = _pack_pm(np.ascontiguousarray(a.T), 8)          # [128, 8, L]
    v = v.reshape(128, 8, NLQ, 2, 512).transpose(0, 2, 3, 1, 4)
    return np.ascontiguousarray(v).astype(BF16)


def _pack_w(w, t=8):
    # [1024, 256] -> [128, 2(ct), 8(ht), 128] partition-major, ct-contiguous
    v = _pack_pm(w, t)                                  # [128, 8, 256]
    v = v.reshape(128, t, 2, 128).transpose(0, 2, 1, 3)
    return np.ascontiguousarray(v).astype(BF16)


def _in_maps(x, y, Wq, Wk, Wv, Wo):
    maps = []
    for core in range(8):
        b, g = core // GP, core % GP
        cs = slice(g * CH, (g + 1) * CH)
        maps.append({
            "xT": _pack_act(x[b]),
            "yT": _pack_act(y[b]),
            "wq": _pack_w(Wq[:, cs] * np.float32(0.125)),
            "wk": _pack_w(Wk[:, cs]),
            "wv": np.ascontiguousarray(_pack_pm(Wv[:, cs], 8)).astype(BF16),
            "wo": np.ascontiguousarray(_pack_pm(Wo[cs, :], 2)).astype(BF16),
        })
    return maps


def _install_ntff_hook():
    """Provide the antenv.axon_hooks shim missing from this container so
    run_bass_kernel_spmd(trace=True) can drive NTFF profiling via ctypes."""
    import sys
    import types
    try:
        from antenv.axon_hooks import get_axon_ntff_profile_hook  # noqa: F401
        return
    except ImportError:
        pass
    from trn_agent_boot.trn_boot import _ntff_profile_via_ctypes
    hook = _ntff_profile_via_ctypes("/opt/axon/libaxon_pjrt.so")
    mod = types.ModuleType("antenv.axon_hooks")
    mod.get_axon_ntff_profile_hook = lambda: hook
    mod.set_axon_ntff_profile_hook = lambda h: None
    sys.modules["antenv.axon_hooks"] = mod


def _run(inputs, trace=False):
    from concourse import bass_utils

    if trace:
        _install_ntff_hook()

    x, y, bias = inputs["x"], inputs["y"], inputs["bias"]
    if np.count_nonzero(np.asarray(bias)):
        raise NotImplementedError("nonzero attention bias not supported")
    nc = _get_nc()
    maps = _in_maps(np.asarray(x, np.float32), np.asarray(y, np.float32),
                    np.asarray(inputs["Wq"], np.float32),
                    np.asarray(inputs["Wk"], np.float32),
                    np.asarray(inputs["Wv"], np.float32),
                    np.asarray(inputs["Wo"], np.float32))
    res = bass_utils.run_bass_kernel_spmd(
        nc, maps, list(range(8)), trace=trace)
    out = np.zeros((B, L, H), np.float32)
    for core in range(8):
        out[core // GP] += res.results[core]["out"].astype(np.float32)
    return out, res


def kernel(**inputs):
    out, _ = _run(inputs, trace=False)
    return out



# revision 49
# speedup vs baseline: 1.0086x; 1.0086x over previous
"""Multi-head attention (B=2, L=2048, H=1024, NH=16) on 8 TRN2 NeuronCores.

Sharding: data-parallel over batch (2) x tensor-parallel over heads (4 groups
of 4 heads).  core = b*4 + g handles batch b, heads [4g, 4g+4).  Wq/Wk/Wv are
split column-wise, Wo row-wise; each core produces a partial [L, H] output
that the host sums per batch (the row-parallel all-reduce done host-side).

Device math (per core), all matmuls bf16 inputs / fp32 PSUM accumulation:
  QT = (Wq*0.125)^T x^T          [256, 2048]  (softmax scale folded into Wq)
  KT = Wk^T y^T                  [256, 2048]
  V  = y Wv                      [2048, 256] stored as V_aug [lk, 4*(64+1)]
                                 with a ones column per head
  per head h, per 1024-wide lq chunk:
    S^T[lk, lq] = KT_h^T QT_h    (contraction d=64)
    P^T = exp(S^T)               ScalarE, PSUM -> SBUF bf16 (no max-subtract:
                                 logits are O(1) by construction)
    O^T_aug[65, lq] = sum_lk V_aug_h^T P^T   (row 64 = softmax denominators)
    O'^T = O^T * broadcast(1/sums)           DVE recip + GpSimd partition bcast
  out[lq, 1024] += O'^T_cat^T Wo  (partial; host sums the 4 head-groups)

Schedule: a zero-matmul warm-up ramps the PE DVFS clock while the first
DMAs land; the startup window runs K ct0 lh0 + V lk 0:1024 + Q ct0 lh0;
the per-lkt exp stream then runs with remaining projections and stage 3
of chunk 0 riding inside it as single-slot hook groups placed by
deadline.  PSUM: 3 rotating psS slots (so an S weight-load is never
gated on the exp that frees its slot) + 1 psO accumulator slot that a
DVE drain frees at each head boundary.  Inputs are host-packed
partition-major so every DMA is 128 contiguous runs; the output is
stored bf16 (the host sums partials in f32).
"""

import numpy as np
import ml_dtypes

B, L, H, NH, D = 2, 2048, 1024, 16, 64
GP = 4            # head-groups (tensor-parallel factor)
CH = H // GP      # 256 local projection cols per core
HL = NH // GP     # 4 local heads
LQ = 1024         # lq chunk size
NLQ = L // LQ
NKT = L // 128    # 16 lk tiles
BF16 = ml_dtypes.bfloat16

_CACHE = {}


def _build():
    import concourse.mybir as mybir
    import concourse.tile as tile
    from concourse import bacc

    dt = mybir.dt
    f32, bf16 = dt.float32, dt.bfloat16
    Exp = mybir.ActivationFunctionType.Exp

    nc = bacc.Bacc("TRN2", target_bir_lowering=False, debug=False, use_seq_codegen=True)
    # all inputs host-packed partition-major so each DMA is 128 long
    # contiguous runs (SP descriptor generation is the startup bottleneck)
    xT = nc.declare_dram_parameter("xT", [128, NLQ, 2, 8, 512], bf16,
                                   isOutput=False)
    yT = nc.declare_dram_parameter("yT", [128, NLQ, 2, 8, 512], bf16,
                                   isOutput=False)
    wq = nc.declare_dram_parameter("wq", [128, 2, 8, 128], bf16,
                                   isOutput=False)
    wk = nc.declare_dram_parameter("wk", [128, 2, 8, 128], bf16,
                                   isOutput=False)
    wv = nc.declare_dram_parameter("wv", [128, 8, CH], bf16, isOutput=False)
    wo = nc.declare_dram_parameter("wo", [128, 2, H], bf16, isOutput=False)
    out = nc.declare_dram_parameter("out", [L, H], bf16, isOutput=True)

    with tile.TileContext(nc) as tc:
        with (
            tc.tile_pool(name="w", bufs=1) as wpool,
            tc.tile_pool(name="acts", bufs=1) as apool,
            tc.tile_pool(name="psA", bufs=3, space="PSUM") as psA,
            tc.tile_pool(name="psO", bufs=1, space="PSUM") as psO,
            tc.tile_pool(name="pt", bufs=8) as ptpool,
            tc.tile_pool(name="oT", bufs=2) as otpool,
            tc.tile_pool(name="sm", bufs=2) as smpool,
            tc.tile_pool(name="osb", bufs=6) as opool,
        ):
            # prefetch the exp activation table while input DMAs run
            dummy = smpool.tile([1, 8], f32, tag="dummy")
            nc.vector.memset(dummy, 0.0)
            nc.scalar.activation(dummy, dummy, Exp)

            # PE warm-up: stream zero matmuls through the array while the
            # input DMAs run, so the tensor engine's DVFS ramp (0.65 ->
            # 2.4 GHz over ~3us of continuous work) completes before the
            # first real projection -- and ends just as its inputs land
            wz = wpool.tile([128, 128], bf16, tag="wz")
            az = wpool.tile([128, 512], bf16, tag="az")
            nc.vector.memset(wz, 0.0)
            nc.vector.memset(az, 0.0)
            psW = psO.tile([128, 512], f32, tag="psO", name="psW")
            for i in range(9):
                nc.tensor.matmul(psW, lhsT=wz, rhs=az,
                                 start=(i == 0), stop=(i == 8))

            # ---- input DMAs, ordered so the PE can run projection work
            # continuously through the DMA-bound startup window: y chunk 0
            # + K weights first (K ct0 + V groups), then x chunk 0 + Q
            # weights (Q ct0 lh0 -> first exp), then the rest ------------
            wk_sb = wpool.tile([128, 2, 8, 128], bf16, tag="wk")
            wq_sb = wpool.tile([128, 2, 8, 128], bf16, tag="wq")
            yT_sb = apool.tile([128, NLQ, 2, 8, 512], bf16, tag="yT")
            xT_sb = apool.tile([128, NLQ, 2, 8, 512], bf16, tag="xT")
            wv_sb = wpool.tile([128, 8, CH], bf16, tag="wv")
            for hp in range(4):
                nc.sync.dma_start(yT_sb[:, 0, 0, 2 * hp:2 * hp + 2],
                                  yT[:, 0, 0, 2 * hp:2 * hp + 2])
                if hp == 0:
                    nc.sync.dma_start(wk_sb[:, 0, 0:4], wk[:, 0, 0:4])
                elif hp == 1:
                    nc.sync.dma_start(wk_sb[:, 0, 4:8], wk[:, 0, 4:8])
            nc.sync.dma_start(wv_sb, wv[:, :, :])
            nc.sync.dma_start(yT_sb[:, 0, 1], yT[:, 0, 1])
            nc.sync.dma_start(xT_sb[:, 0, 0], xT[:, 0, 0])
            nc.sync.dma_start(wq_sb[:, 0], wq[:, 0])
            nc.sync.dma_start(xT_sb[:, 0, 1], xT[:, 0, 1])
            for sl in range(2):
                nc.sync.dma_start(yT_sb[:, 1, sl], yT[:, 1, sl])
            nc.sync.dma_start(wk_sb[:, 1], wk[:, 1])
            nc.sync.dma_start(wq_sb[:, 1], wq[:, 1])
            for sl in range(2):
                nc.sync.dma_start(xT_sb[:, 1, sl], xT[:, 1, sl])
            wo_sb = wpool.tile([128, 2, H], bf16, tag="wo")
            nc.sync.dma_start(wo_sb, wo[:, :, :])

            qT_sb = apool.tile([128, 2, L], bf16, tag="qT")
            kT_sb = apool.tile([128, 2, L], bf16, tag="kT")
            vaug_sb = apool.tile([128, NKT, HL * 65], bf16, tag="vaug")

            def proj_group(w_sb, act_sb, dst, ct, lh, sl):
                # dst[:, ct, lh*LQ+sl*512 : +512] via one 8-matmul psum group
                ps = psA.tile([128, LQ], f32, tag="psA")
                off = lh * LQ + sl * 512
                for ht in range(8):
                    nc.tensor.matmul(
                        ps[:, 0:512],
                        lhsT=w_sb[:, ct, ht, :],
                        rhs=act_sb[:, lh, sl, ht, :],
                        start=(ht == 0), stop=(ht == 7),
                    )
                nc.vector.tensor_copy(dst[:, ct, off:off + 512], ps[:, 0:512])

            def v_group(lkt):
                # one lk tile of V_aug[lk, 4*(64+1)] bf16 (+ones col per head)
                psv = psA.tile([128, LQ], f32, tag="psA")
                for ht in range(8):
                    nc.tensor.matmul(
                        psv[:, :CH],
                        lhsT=yT_sb[:, lkt // 8, (lkt % 8) // 4, ht,
                                   (lkt % 4) * 128:(lkt % 4 + 1) * 128],
                        rhs=wv_sb[:, ht, :],
                        start=(ht == 0), stop=(ht == 7),
                    )
                vh = vaug_sb[:, lkt, :].rearrange("p (h e) -> p h e", h=HL)
                nc.vector.tensor_copy(
                    vh[:, :, 0:64],
                    psv[:, :CH].rearrange("p (h e) -> p h e", h=HL))
                nc.vector.memset(vh[:, :, 64], 1.0)

            def s3_piece(ci, oT_sb, mt, pool=None, act_copy=False):
                # both 512-wide output halves in one psum tile -> one copy
                # and one full-width contiguous store (amortizes the
                # slot->copy->store latency that paces the tail)
                pool = pool if pool is not None else psO
                pso = pool.tile([128, LQ], f32,
                                tag="psO" if pool is psO else "psA")
                for nt in range(2):
                    for kt in range(2):
                        nc.tensor.matmul(
                            pso[:, nt * 512:(nt + 1) * 512],
                            lhsT=oT_sb[:, kt, mt * 128:(mt + 1) * 128],
                            rhs=wo_sb[:, kt, nt * 512:(nt + 1) * 512],
                            start=(kt == 0), stop=(kt == 1),
                        )
                osb = opool.tile([128, LQ], bf16, tag="osb")
                if act_copy:
                    nc.scalar.copy(osb, pso)
                else:
                    nc.vector.tensor_copy(osb, pso)
                nc.sync.dma_start(
                    out[ci * LQ + mt * 128:ci * LQ + (mt + 1) * 128, :],
                    osb)

            def s3q(ci, oT_sb, mt, nt, act_copy=False):
                # one 512-wide quarter of a stage-3 piece as two ~213ns PE
                # thunks for consecutive hook slots: fine-grained filler
                # that keeps the PE streaming (and clocked up) through the
                # ACT-paced heads
                cell = {}

                def t0():
                    cell["ps"] = psA.tile(
                        [128, 512], f32, tag="psA",
                        name=f"s3q{ci}_{mt}_{nt}")
                    nc.tensor.matmul(
                        cell["ps"],
                        lhsT=oT_sb[:, 0, mt * 128:(mt + 1) * 128],
                        rhs=wo_sb[:, 0, nt * 512:(nt + 1) * 512],
                        start=True, stop=False,
                    )

                def t1():
                    ps = cell["ps"]
                    nc.tensor.matmul(
                        ps,
                        lhsT=oT_sb[:, 1, mt * 128:(mt + 1) * 128],
                        rhs=wo_sb[:, 1, nt * 512:(nt + 1) * 512],
                        start=False, stop=True,
                    )
                    osb = opool.tile([128, 512], bf16, tag="osb",
                                     name=f"osbq{ci}_{mt}_{nt}")
                    if act_copy:
                        nc.scalar.copy(osb, ps)
                    else:
                        nc.vector.tensor_copy(osb, ps)
                    nc.sync.dma_start(
                        out[ci * LQ + mt * 128:ci * LQ + (mt + 1) * 128,
                            nt * 512:(nt + 1) * 512],
                        osb)
                return t0, t1

            def emit_S(ci, h, lkt):
                po, ct2 = h % 2, h // 2
                psS = psA.tile([128, LQ], f32, tag="psA")
                for sl in range(LQ // 512):
                    nc.tensor.matmul(
                        psS[:, sl * 512:(sl + 1) * 512],
                        lhsT=kT_sb[64 * po:64 * po + 64, ct2,
                                   lkt * 128:(lkt + 1) * 128],
                        rhs=qT_sb[64 * po:64 * po + 64, ct2,
                                  ci * LQ + sl * 512:
                                  ci * LQ + (sl + 1) * 512],
                        start=True, stop=True,
                    )
                return psS

            pipe = {}

            def s2(ci, h, oT_sb, extra=None, nxt=None, nq=2):
                # per lk tile: S^T matmuls -> exp -> O^T accumulation.  S
                # matmuls run one lkt ahead of the O matmuls (and prefetch
                # across head boundaries via `nxt`) so exp(k+1)'s input is
                # ready the moment ScalarE finishes exp(k).
                po, ct2 = h % 2, h // 2
                psO_h = psO.tile([128, LQ], f32, tag="psO")
                psS = pipe.pop("psS", None)
                if psS is None:
                    psS = emit_S(ci, h, 0)
                for lkt in range(NKT):
                    pt = ptpool.tile([128, LQ], bf16, tag="pt")
                    nc.scalar.activation(pt, psS, Exp)
                    if lkt + 1 < NKT:
                        psS = emit_S(ci, h, lkt + 1)
                    elif nxt is not None:
                        pipe["psS"] = emit_S(nxt[0], nxt[1], 0)
                    if extra is not None:
                        extra(lkt)
                    for sl in range(LQ // 512):
                        nc.tensor.matmul(
                            psO_h[0:65, sl * 512:(sl + 1) * 512],
                            lhsT=vaug_sb[:, lkt, h * 65:(h + 1) * 65],
                            rhs=pt[:, sl * 512:(sl + 1) * 512],
                            start=(lkt == 0), stop=(lkt == NKT - 1),
                        )
                # DVE drains free the psO banks for the next head (sums
                # via tensor_copy: partition-remapping 64 -> 0); normalize
                # then runs from SBUF in nq lq pieces.  The last head uses
                # nq=4 with interleaved quarter-drains so the tail's first
                # stage-3 pieces unlock as soon as possible.
                sums = smpool.tile([1, LQ], f32, tag="sums")
                oc = smpool.tile([64, LQ], f32, tag="oc")
                w = LQ // nq
                nc.vector.tensor_copy(sums, psO_h[64:65, :])
                if nq == 2:
                    nc.vector.tensor_copy(oc, psO_h[0:64, :])
                for hs in range(nq):
                    c = slice(hs * w, (hs + 1) * w)
                    recip = smpool.tile([1, 512], f32, tag="recip")
                    nc.vector.reciprocal_approx_fast(recip[:, :w],
                                                     sums[:, c])
                    bcast = smpool.tile([64, 512], f32, tag="bcast")
                    nc.gpsimd.partition_broadcast(bcast[:, :w], recip[:, :w])
                    if nq != 2:
                        nc.vector.tensor_copy(oc[:, c], psO_h[0:64, c])
                    nc.vector.tensor_mul(
                        oT_sb[64 * po:64 * po + 64, ct2, c], oc[:, c],
                        bcast[:, :w])

            # ---- emission order: pack the DMA-bound startup window with
            # all the projection work chunk-0 y/x can feed (K ct0 lh0, V
            # lk 0:1024, Q ct0 lh0), then run a continuous per-lkt exp
            # stream; remaining projections and stage 3 ride inside the
            # stream as per-deadline hook work ----------------------------
            oT = [otpool.tile([128, 2, LQ], bf16, tag="oT", name=f"oT{i}")
                  for i in range(NLQ)]
            proj_group(wk_sb, yT_sb, kT_sb, 0, 0, 0)
            for j in range(4):
                v_group(j)
            proj_group(wk_sb, yT_sb, kT_sb, 0, 0, 1)
            for j in range(4, 8):
                v_group(j)
            proj_group(wq_sb, xT_sb, qT_sb, 0, 0, 0)
            proj_group(wq_sb, xT_sb, qT_sb, 0, 0, 1)

            def make_hook(sched):
                # sched: lkt -> list of thunks
                def hook(lkt):
                    for job in sched.get(lkt, ()):
                        job()
                return hook

            vj = [(lambda j=j: v_group(j)) for j in range(NKT)]
            pj = lambda w, a, d, ct, lh, sl: (  # noqa: E731
                lambda: proj_group(w, a, d, ct, lh, sl))

            def sched(items):
                # items: (slot, thunk) pairs -> hook.  Hooks are whole
                # psum groups in a single slot: fewer polluted iterations
                # keeps the S/O weight-load pipeline clean.
                d = {}
                for s, t in items:
                    d.setdefault(s, []).append(t)
                return make_hook(d)

            def s3h(ci, mt, nt, act_copy=False):
                a, b = s3q(ci, oT[ci], mt, nt, act_copy)
                return lambda: (a(), b())

            # head 0 carries the second half of V (v_group(j) just before
            # the O-mms of lkt j) plus the K ct0 lh1 groups (whose lk
            # tiles are first consumed from lkt 8 on)
            h0_sched = {j - 1: [vj[j]] for j in range(8, NKT)}
            h0_sched[2] = [pj(wk_sb, yT_sb, kT_sb, 0, 1, 0)]
            h0_sched[5] = [pj(wk_sb, yT_sb, kT_sb, 0, 1, 1)]
            h0_hook = make_hook(h0_sched)
            # per-deadline balance: Q ct1 lh0 and K(1,0,0) are consumed
            # right after h1 (S prefetch of head 2); the later K ct1
            # pieces are consumed lk-tile by lk-tile through h2
            h1_hook = sched([
                (0, pj(wq_sb, xT_sb, qT_sb, 1, 0, 0)),
                (5, pj(wq_sb, xT_sb, qT_sb, 1, 0, 1)),
                (10, pj(wk_sb, yT_sb, kT_sb, 1, 0, 0))])
            h2_hook = sched([
                (0, pj(wk_sb, yT_sb, kT_sb, 1, 0, 1)),
                (5, pj(wk_sb, yT_sb, kT_sb, 1, 1, 0)),
                (9, pj(wk_sb, yT_sb, kT_sb, 1, 1, 1))])
            h3_hook = sched([
                (0, pj(wq_sb, xT_sb, qT_sb, 0, 1, 0)),
                (6, pj(wq_sb, xT_sb, qT_sb, 0, 1, 1))])
            h4_hook = sched([
                (0, pj(wq_sb, xT_sb, qT_sb, 1, 1, 0)),
                (6, pj(wq_sb, xT_sb, qT_sb, 1, 1, 1))])
            # chunk-0 stage 3 smears over the ACT-paced chunk-1 heads
            # (oT[0] is final after h3), one 512-wide half per slot
            h5_hook = sched([
                (1, s3h(0, 0, 0)), (4, s3h(0, 0, 1)), (7, s3h(0, 1, 0)),
                (10, s3h(0, 1, 1)), (13, s3h(0, 2, 0))])
            h6_hook = sched([
                (1, s3h(0, 2, 1)), (4, s3h(0, 3, 0)), (7, s3h(0, 3, 1)),
                (10, s3h(0, 4, 0)), (13, s3h(0, 4, 1))])
            h7_hook = sched([
                (0, s3h(0, 5, 0)), (4, s3h(0, 5, 1)), (8, s3h(0, 6, 0))])

            s2(0, 0, oT[0], extra=h0_hook, nxt=(0, 1))
            s2(0, 1, oT[0], extra=h1_hook, nxt=(0, 2))
            s2(0, 2, oT[0], extra=h2_hook, nxt=(0, 3))
            s2(0, 3, oT[0], extra=h3_hook, nxt=(1, 0))
            s2(1, 0, oT[1], extra=h4_hook, nxt=(1, 1))
            s2(1, 1, oT[1], extra=h5_hook, nxt=(1, 2))
            s2(1, 2, oT[1], extra=h6_hook, nxt=(1, 3))
            s2(1, 3, oT[1], extra=h7_hook, nq=4)
            # tail: the remaining chunk-0 stage-3 pieces fill the PE gap
            # while h7's first normalize quarter-chain runs, then chunk-1
            # stage 3 drains.  The first pieces copy out via ScalarE (the
            # DVE is still working through the normalize); later ones DVE.
            s3h(0, 6, 1, act_copy=True)()
            s3h(0, 7, 0, act_copy=True)()
            s3h(0, 7, 1, act_copy=True)()
            for mt in range(LQ // 128):
                s3_piece(1, oT[1], mt, pool=(psA if mt < 3 or mt % 2 else psO),
                         act_copy=(mt < 4))
    nc.compile()
    return nc


def _get_nc():
    if "nc" not in _CACHE:
        _CACHE["nc"] = _build()
    return _CACHE["nc"]


def _pack_pm(a, t):
    # [t*128, N] -> [128, t, N] partition-major
    return a.reshape(t, 128, -1).transpose(1, 0, 2)


def _pack_act(a):
    # x[b] [L, H] -> xT packed [128, NLQ(lh), 2(sl), 8(t), 512] bf16
    v = _pack_pm(np.ascontiguousarray(a.T), 8)          # [128, 8, L]
    v = v.reshape(128, 8, NLQ, 2, 512).transpose(0, 2, 3, 1, 4)
    return np.ascontiguousarray(v).astype(BF16)


def _pack_w(w, t=8):
    # [1024, 256] -> [128, 2(ct), 8(ht), 128] partition-major, ct-contiguous
    v = _pack_pm(w, t)                                  # [128, 8, 256]
    v = v.reshape(128, t, 2, 128).transpose(0, 2, 1, 3)
    return np.ascontiguousarray(v).astype(BF16)


def _in_maps(x, y, Wq, Wk, Wv, Wo):
    maps = []
    for core in range(8):
        b, g = core // GP, core % GP
        cs = slice(g * CH, (g + 1) * CH)
        maps.append({
            "xT": _pack_act(x[b]),
            "yT": _pack_act(y[b]),
            "wq": _pack_w(Wq[:, cs] * np.float32(0.125)),
            "wk": _pack_w(Wk[:, cs]),
            "wv": np.ascontiguousarray(_pack_pm(Wv[:, cs], 8)).astype(BF16),
            "wo": np.ascontiguousarray(_pack_pm(Wo[cs, :], 2)).astype(BF16),
        })
    return maps


def _install_ntff_hook():
    """Provide the antenv.axon_hooks shim missing from this container so
    run_bass_kernel_spmd(trace=True) can drive NTFF profiling via ctypes."""
    import sys
    import types
    try:
        from antenv.axon_hooks import get_axon_ntff_profile_hook  # noqa: F401
        return
    except ImportError:
        pass
    from trn_agent_boot.trn_boot import _ntff_profile_via_ctypes
    hook = _ntff_profile_via_ctypes("/opt/axon/libaxon_pjrt.so")
    mod = types.ModuleType("antenv.axon_hooks")
    mod.get_axon_ntff_profile_hook = lambda: hook
    mod.set_axon_ntff_profile_hook = lambda h: None
    sys.modules["antenv.axon_hooks"] = mod


def _run(inputs, trace=False):
    from concourse import bass_utils

    if trace:
        _install_ntff_hook()

    x, y, bias = inputs["x"], inputs["y"], inputs["bias"]
    if np.count_nonzero(np.asarray(bias)):
        raise NotImplementedError("nonzero attention bias not supported")
    nc = _get_nc()
    maps = _in_maps(np.asarray(x, np.float32), np.asarray(y, np.float32),
                    np.asarray(inputs["Wq"], np.float32),
                    np.asarray(inputs["Wk"], np.float32),
                    np.asarray(inputs["Wv"], np.float32),
                    np.asarray(inputs["Wo"], np.float32))
    res = bass_utils.run_bass_kernel_spmd(
        nc, maps, list(range(8)), trace=trace)
    out = np.zeros((B, L, H), np.float32)
    for core in range(8):
        out[core // GP] += res.results[core]["out"].astype(np.float32)
    return out, res


def kernel(**inputs):
    out, _ = _run(inputs, trace=False)
    return out



# revision 50
# speedup vs baseline: 1.0096x; 1.0009x over previous
"""Multi-head attention (B=2, L=2048, H=1024, NH=16) on 8 TRN2 NeuronCores.

Sharding: data-parallel over batch (2) x tensor-parallel over heads (4 groups
of 4 heads).  core = b*4 + g handles batch b, heads [4g, 4g+4).  Wq/Wk/Wv are
split column-wise, Wo row-wise; each core produces a partial [L, H] output
that the host sums per batch (the row-parallel all-reduce done host-side).

Device math (per core), all matmuls bf16 inputs / fp32 PSUM accumulation:
  QT = (Wq*0.125)^T x^T          [256, 2048]  (softmax scale folded into Wq)
  KT = Wk^T y^T                  [256, 2048]
  V  = y Wv                      [2048, 256] stored as V_aug [lk, 4*(64+1)]
                                 with a ones column per head
  per head h, per 1024-wide lq chunk:
    S^T[lk, lq] = KT_h^T QT_h    (contraction d=64)
    P^T = exp(S^T)               ScalarE, PSUM -> SBUF bf16 (no max-subtract:
                                 logits are O(1) by construction)
    O^T_aug[65, lq] = sum_lk V_aug_h^T P^T   (row 64 = softmax denominators)
    O'^T = O^T * broadcast(1/sums)           DVE recip + GpSimd partition bcast
  out[lq, 1024] += O'^T_cat^T Wo  (partial; host sums the 4 head-groups)

Schedule: a zero-matmul warm-up ramps the PE DVFS clock while the first
DMAs land; the startup window runs K ct0 lh0 + V lk 0:1024 + Q ct0 lh0;
the per-lkt exp stream then runs with remaining projections and stage 3
of chunk 0 riding inside it as single-slot hook groups placed by
deadline.  PSUM: 3 rotating psS slots (so an S weight-load is never
gated on the exp that frees its slot) + 1 psO accumulator slot that a
DVE drain frees at each head boundary.  Inputs are host-packed
partition-major so every DMA is 128 contiguous runs; the output is
stored bf16 (the host sums partials in f32).
"""

import numpy as np
import ml_dtypes

B, L, H, NH, D = 2, 2048, 1024, 16, 64
GP = 4            # head-groups (tensor-parallel factor)
CH = H // GP      # 256 local projection cols per core
HL = NH // GP     # 4 local heads
LQ = 1024         # lq chunk size
NLQ = L // LQ
NKT = L // 128    # 16 lk tiles
BF16 = ml_dtypes.bfloat16

_CACHE = {}


def _build():
    import concourse.mybir as mybir
    import concourse.tile as tile
    from concourse import bacc

    dt = mybir.dt
    f32, bf16 = dt.float32, dt.bfloat16
    Exp = mybir.ActivationFunctionType.Exp

    nc = bacc.Bacc("TRN2", target_bir_lowering=False, debug=False, use_seq_codegen=True)
    # all inputs host-packed partition-major so each DMA is 128 long
    # contiguous runs (SP descriptor generation is the startup bottleneck)
    xT = nc.declare_dram_parameter("xT", [128, NLQ, 2, 8, 512], bf16,
                                   isOutput=False)
    yT = nc.declare_dram_parameter("yT", [128, NLQ, 2, 8, 512], bf16,
                                   isOutput=False)
    wq = nc.declare_dram_parameter("wq", [128, 2, 8, 128], bf16,
                                   isOutput=False)
    wk = nc.declare_dram_parameter("wk", [128, 2, 8, 128], bf16,
                                   isOutput=False)
    wv = nc.declare_dram_parameter("wv", [128, 8, CH], bf16, isOutput=False)
    wo = nc.declare_dram_parameter("wo", [128, 2, H], bf16, isOutput=False)
    out = nc.declare_dram_parameter("out", [L, H], bf16, isOutput=True)

    with tile.TileContext(nc) as tc:
        with (
            tc.tile_pool(name="w", bufs=1) as wpool,
            tc.tile_pool(name="acts", bufs=1) as apool,
            tc.tile_pool(name="psA", bufs=3, space="PSUM") as psA,
            tc.tile_pool(name="psO", bufs=1, space="PSUM") as psO,
            tc.tile_pool(name="pt", bufs=12) as ptpool,
            tc.tile_pool(name="oT", bufs=2) as otpool,
            tc.tile_pool(name="sm", bufs=2) as smpool,
            tc.tile_pool(name="osb", bufs=6) as opool,
        ):
            # prefetch the exp activation table while input DMAs run
            dummy = smpool.tile([1, 8], f32, tag="dummy")
            nc.vector.memset(dummy, 0.0)
            nc.scalar.activation(dummy, dummy, Exp)

            # PE warm-up: stream zero matmuls through the array while the
            # input DMAs run, so the tensor engine's DVFS ramp (0.65 ->
            # 2.4 GHz over ~3us of continuous work) completes before the
            # first real projection -- and ends just as its inputs land
            wz = wpool.tile([128, 128], bf16, tag="wz")
            az = wpool.tile([128, 512], bf16, tag="az")
            nc.vector.memset(wz, 0.0)
            nc.vector.memset(az, 0.0)
            psW = psO.tile([128, 512], f32, tag="psO", name="psW")
            for i in range(9):
                nc.tensor.matmul(psW, lhsT=wz, rhs=az,
                                 start=(i == 0), stop=(i == 8))

            # ---- input DMAs, ordered so the PE can run projection work
            # continuously through the DMA-bound startup window: y chunk 0
            # + K weights first (K ct0 + V groups), then x chunk 0 + Q
            # weights (Q ct0 lh0 -> first exp), then the rest ------------
            wk_sb = wpool.tile([128, 2, 8, 128], bf16, tag="wk")
            wq_sb = wpool.tile([128, 2, 8, 128], bf16, tag="wq")
            yT_sb = apool.tile([128, NLQ, 2, 8, 512], bf16, tag="yT")
            xT_sb = apool.tile([128, NLQ, 2, 8, 512], bf16, tag="xT")
            wv_sb = wpool.tile([128, 8, CH], bf16, tag="wv")
            for hp in range(4):
                nc.sync.dma_start(yT_sb[:, 0, 0, 2 * hp:2 * hp + 2],
                                  yT[:, 0, 0, 2 * hp:2 * hp + 2])
                if hp == 0:
                    nc.sync.dma_start(wk_sb[:, 0, 0:4], wk[:, 0, 0:4])
                elif hp == 1:
                    nc.sync.dma_start(wk_sb[:, 0, 4:8], wk[:, 0, 4:8])
            nc.sync.dma_start(wv_sb, wv[:, :, :])
            nc.sync.dma_start(yT_sb[:, 0, 1], yT[:, 0, 1])
            nc.sync.dma_start(xT_sb[:, 0, 0], xT[:, 0, 0])
            nc.sync.dma_start(wq_sb[:, 0], wq[:, 0])
            nc.sync.dma_start(xT_sb[:, 0, 1], xT[:, 0, 1])
            for sl in range(2):
                nc.sync.dma_start(yT_sb[:, 1, sl], yT[:, 1, sl])
            nc.sync.dma_start(wk_sb[:, 1], wk[:, 1])
            nc.sync.dma_start(wq_sb[:, 1], wq[:, 1])
            for sl in range(2):
                nc.sync.dma_start(xT_sb[:, 1, sl], xT[:, 1, sl])
            wo_sb = wpool.tile([128, 2, H], bf16, tag="wo")
            nc.sync.dma_start(wo_sb, wo[:, :, :])

            qT_sb = apool.tile([128, 2, L], bf16, tag="qT")
            kT_sb = apool.tile([128, 2, L], bf16, tag="kT")
            vaug_sb = apool.tile([128, NKT, HL * 65], bf16, tag="vaug")

            def proj_group(w_sb, act_sb, dst, ct, lh, sl):
                # dst[:, ct, lh*LQ+sl*512 : +512] via one 8-matmul psum group
                ps = psA.tile([128, LQ], f32, tag="psA")
                off = lh * LQ + sl * 512
                for ht in range(8):
                    nc.tensor.matmul(
                        ps[:, 0:512],
                        lhsT=w_sb[:, ct, ht, :],
                        rhs=act_sb[:, lh, sl, ht, :],
                        start=(ht == 0), stop=(ht == 7),
                    )
                nc.vector.tensor_copy(dst[:, ct, off:off + 512], ps[:, 0:512])

            def v_group(lkt):
                # one lk tile of V_aug[lk, 4*(64+1)] bf16 (+ones col per head)
                psv = psA.tile([128, LQ], f32, tag="psA")
                for ht in range(8):
                    nc.tensor.matmul(
                        psv[:, :CH],
                        lhsT=yT_sb[:, lkt // 8, (lkt % 8) // 4, ht,
                                   (lkt % 4) * 128:(lkt % 4 + 1) * 128],
                        rhs=wv_sb[:, ht, :],
                        start=(ht == 0), stop=(ht == 7),
                    )
                vh = vaug_sb[:, lkt, :].rearrange("p (h e) -> p h e", h=HL)
                nc.vector.tensor_copy(
                    vh[:, :, 0:64],
                    psv[:, :CH].rearrange("p (h e) -> p h e", h=HL))
                nc.vector.memset(vh[:, :, 64], 1.0)

            def s3_piece(ci, oT_sb, mt, pool=None, act_copy=False):
                # both 512-wide output halves in one psum tile -> one copy
                # and one full-width contiguous store (amortizes the
                # slot->copy->store latency that paces the tail)
                pool = pool if pool is not None else psO
                pso = pool.tile([128, LQ], f32,
                                tag="psO" if pool is psO else "psA")
                for nt in range(2):
                    for kt in range(2):
                        nc.tensor.matmul(
                            pso[:, nt * 512:(nt + 1) * 512],
                            lhsT=oT_sb[:, kt, mt * 128:(mt + 1) * 128],
                            rhs=wo_sb[:, kt, nt * 512:(nt + 1) * 512],
                            start=(kt == 0), stop=(kt == 1),
                        )
                osb = opool.tile([128, LQ], bf16, tag="osb")
                if act_copy:
                    nc.scalar.copy(osb, pso)
                else:
                    nc.vector.tensor_copy(osb, pso)
                nc.sync.dma_start(
                    out[ci * LQ + mt * 128:ci * LQ + (mt + 1) * 128, :],
                    osb)

            def s3q(ci, oT_sb, mt, nt, act_copy=False):
                # one 512-wide quarter of a stage-3 piece as two ~213ns PE
                # thunks for consecutive hook slots: fine-grained filler
                # that keeps the PE streaming (and clocked up) through the
                # ACT-paced heads
                cell = {}

                def t0():
                    cell["ps"] = psA.tile(
                        [128, 512], f32, tag="psA",
                        name=f"s3q{ci}_{mt}_{nt}")
                    nc.tensor.matmul(
                        cell["ps"],
                        lhsT=oT_sb[:, 0, mt * 128:(mt + 1) * 128],
                        rhs=wo_sb[:, 0, nt * 512:(nt + 1) * 512],
                        start=True, stop=False,
                    )

                def t1():
                    ps = cell["ps"]
                    nc.tensor.matmul(
                        ps,
                        lhsT=oT_sb[:, 1, mt * 128:(mt + 1) * 128],
                        rhs=wo_sb[:, 1, nt * 512:(nt + 1) * 512],
                        start=False, stop=True,
                    )
                    osb = opool.tile([128, 512], bf16, tag="osb",
                                     name=f"osbq{ci}_{mt}_{nt}")
                    if act_copy:
                        nc.scalar.copy(osb, ps)
                    else:
                        nc.vector.tensor_copy(osb, ps)
                    nc.sync.dma_start(
                        out[ci * LQ + mt * 128:ci * LQ + (mt + 1) * 128,
                            nt * 512:(nt + 1) * 512],
                        osb)
                return t0, t1

            def emit_S(ci, h, lkt):
                po, ct2 = h % 2, h // 2
                psS = psA.tile([128, LQ], f32, tag="psA")
                for sl in range(LQ // 512):
                    nc.tensor.matmul(
                        psS[:, sl * 512:(sl + 1) * 512],
                        lhsT=kT_sb[64 * po:64 * po + 64, ct2,
                                   lkt * 128:(lkt + 1) * 128],
                        rhs=qT_sb[64 * po:64 * po + 64, ct2,
                                  ci * LQ + sl * 512:
                                  ci * LQ + (sl + 1) * 512],
                        start=True, stop=True,
                    )
                return psS

            pipe = {}

            def s2(ci, h, oT_sb, extra=None, nxt=None, nq=2):
                # per lk tile: S^T matmuls -> exp -> O^T accumulation.  S
                # matmuls run one lkt ahead of the O matmuls (and prefetch
                # across head boundaries via `nxt`) so exp(k+1)'s input is
                # ready the moment ScalarE finishes exp(k).
                po, ct2 = h % 2, h // 2
                psO_h = psO.tile([128, LQ], f32, tag="psO")
                psS = pipe.pop("psS", None)
                if psS is None:
                    psS = emit_S(ci, h, 0)
                for lkt in range(NKT):
                    pt = ptpool.tile([128, LQ], bf16, tag="pt")
                    nc.scalar.activation(pt, psS, Exp)
                    if lkt + 1 < NKT:
                        psS = emit_S(ci, h, lkt + 1)
                    elif nxt is not None:
                        pipe["psS"] = emit_S(nxt[0], nxt[1], 0)
                    if extra is not None:
                        extra(lkt)
                    for sl in range(LQ // 512):
                        nc.tensor.matmul(
                            psO_h[0:65, sl * 512:(sl + 1) * 512],
                            lhsT=vaug_sb[:, lkt, h * 65:(h + 1) * 65],
                            rhs=pt[:, sl * 512:(sl + 1) * 512],
                            start=(lkt == 0), stop=(lkt == NKT - 1),
                        )
                # DVE drains free the psO banks for the next head (sums
                # via tensor_copy: partition-remapping 64 -> 0); normalize
                # then runs from SBUF in nq lq pieces.  The last head uses
                # nq=4 with interleaved quarter-drains so the tail's first
                # stage-3 pieces unlock as soon as possible.
                sums = smpool.tile([1, LQ], f32, tag="sums")
                oc = smpool.tile([64, LQ], f32, tag="oc")
                w = LQ // nq
                nc.vector.tensor_copy(sums, psO_h[64:65, :])
                if nq == 2:
                    nc.vector.tensor_copy(oc, psO_h[0:64, :])
                for hs in range(nq):
                    c = slice(hs * w, (hs + 1) * w)
                    recip = smpool.tile([1, 512], f32, tag="recip")
                    nc.vector.reciprocal_approx_fast(recip[:, :w],
                                                     sums[:, c])
                    bcast = smpool.tile([64, 512], f32, tag="bcast")
                    nc.gpsimd.partition_broadcast(bcast[:, :w], recip[:, :w])
                    if nq != 2:
                        nc.vector.tensor_copy(oc[:, c], psO_h[0:64, c])
                    nc.vector.tensor_mul(
                        oT_sb[64 * po:64 * po + 64, ct2, c], oc[:, c],
                        bcast[:, :w])

            # ---- emission order: pack the DMA-bound startup window with
            # all the projection work chunk-0 y/x can feed (K ct0 lh0, V
            # lk 0:1024, Q ct0 lh0), then run a continuous per-lkt exp
            # stream; remaining projections and stage 3 ride inside the
            # stream as per-deadline hook work ----------------------------
            oT = [otpool.tile([128, 2, LQ], bf16, tag="oT", name=f"oT{i}")
                  for i in range(NLQ)]
            proj_group(wk_sb, yT_sb, kT_sb, 0, 0, 0)
            for j in range(4):
                v_group(j)
            proj_group(wk_sb, yT_sb, kT_sb, 0, 0, 1)
            for j in range(4, 8):
                v_group(j)
            proj_group(wq_sb, xT_sb, qT_sb, 0, 0, 0)
            proj_group(wq_sb, xT_sb, qT_sb, 0, 0, 1)

            def make_hook(sched):
                # sched: lkt -> list of thunks
                def hook(lkt):
                    for job in sched.get(lkt, ()):
                        job()
                return hook

            vj = [(lambda j=j: v_group(j)) for j in range(NKT)]
            pj = lambda w, a, d, ct, lh, sl: (  # noqa: E731
                lambda: proj_group(w, a, d, ct, lh, sl))

            def sched(items):
                # items: (slot, thunk) pairs -> hook.  Hooks are whole
                # psum groups in a single slot: fewer polluted iterations
                # keeps the S/O weight-load pipeline clean.
                d = {}
                for s, t in items:
                    d.setdefault(s, []).append(t)
                return make_hook(d)

            def s3h(ci, mt, nt, act_copy=False):
                a, b = s3q(ci, oT[ci], mt, nt, act_copy)
                return lambda: (a(), b())

            # head 0 carries the second half of V (v_group(j) just before
            # the O-mms of lkt j) plus the K ct0 lh1 groups (whose lk
            # tiles are first consumed from lkt 8 on)
            h0_sched = {j - 1: [vj[j]] for j in range(8, NKT)}
            h0_sched[2] = [pj(wk_sb, yT_sb, kT_sb, 0, 1, 0)]
            h0_sched[5] = [pj(wk_sb, yT_sb, kT_sb, 0, 1, 1)]
            h0_hook = make_hook(h0_sched)
            # per-deadline balance: Q ct1 lh0 and K(1,0,0) are consumed
            # right after h1 (S prefetch of head 2); the later K ct1
            # pieces are consumed lk-tile by lk-tile through h2
            h1_hook = sched([
                (0, pj(wq_sb, xT_sb, qT_sb, 1, 0, 0)),
                (5, pj(wq_sb, xT_sb, qT_sb, 1, 0, 1)),
                (10, pj(wk_sb, yT_sb, kT_sb, 1, 0, 0))])
            h2_hook = sched([
                (0, pj(wk_sb, yT_sb, kT_sb, 1, 0, 1)),
                (5, pj(wk_sb, yT_sb, kT_sb, 1, 1, 0)),
                (9, pj(wk_sb, yT_sb, kT_sb, 1, 1, 1))])
            h3_hook = sched([
                (0, pj(wq_sb, xT_sb, qT_sb, 0, 1, 0)),
                (6, pj(wq_sb, xT_sb, qT_sb, 0, 1, 1))])
            h4_hook = sched([
                (0, pj(wq_sb, xT_sb, qT_sb, 1, 1, 0)),
                (6, pj(wq_sb, xT_sb, qT_sb, 1, 1, 1))])
            # chunk-0 stage 3 smears over the ACT-paced chunk-1 heads
            # (oT[0] is final after h3), one 512-wide half per slot
            h5_hook = sched([
                (1, s3h(0, 0, 0)), (4, s3h(0, 0, 1)), (7, s3h(0, 1, 0)),
                (10, s3h(0, 1, 1)), (13, s3h(0, 2, 0))])
            h6_hook = sched([
                (1, s3h(0, 2, 1)), (4, s3h(0, 3, 0)), (7, s3h(0, 3, 1)),
                (10, s3h(0, 4, 0)), (13, s3h(0, 4, 1))])
            h7_hook = sched([
                (0, s3h(0, 5, 0)), (4, s3h(0, 5, 1)), (8, s3h(0, 6, 0))])

            s2(0, 0, oT[0], extra=h0_hook, nxt=(0, 1))
            s2(0, 1, oT[0], extra=h1_hook, nxt=(0, 2))
            s2(0, 2, oT[0], extra=h2_hook, nxt=(0, 3))
            s2(0, 3, oT[0], extra=h3_hook, nxt=(1, 0))
            s2(1, 0, oT[1], extra=h4_hook, nxt=(1, 1))
            s2(1, 1, oT[1], extra=h5_hook, nxt=(1, 2))
            s2(1, 2, oT[1], extra=h6_hook, nxt=(1, 3))
            s2(1, 3, oT[1], extra=h7_hook, nq=4)
            # tail: the remaining chunk-0 stage-3 pieces fill the PE gap
            # while h7's first normalize quarter-chain runs, then chunk-1
            # stage 3 drains.  The first pieces copy out via ScalarE (the
            # DVE is still working through the normalize); later ones DVE.
            s3h(0, 6, 1, act_copy=True)()
            s3h(0, 7, 0, act_copy=True)()
            s3h(0, 7, 1, act_copy=True)()
            for mt in range(LQ // 128):
                s3_piece(1, oT[1], mt, pool=(psA if mt < 3 or mt % 2 else psO),
                         act_copy=(mt < 4))
    nc.compile()
    return nc


def _get_nc():
    if "nc" not in _CACHE:
        _CACHE["nc"] = _build()
    return _CACHE["nc"]


def _pack_pm(a, t):
    # [t*128, N] -> [128, t, N] partition-major
    return a.reshape(t, 128, -1).transpose(1, 0, 2)


def _pack_act(a):
    # x[b] [L, H] -> xT packed [128, NLQ(lh), 2(sl), 8(t), 512] bf16
    v = _pack_pm(np.ascontiguousarray(a.T), 8)          # [128, 8, L]
    v = v.reshape(128, 8, NLQ, 2, 512).transpose(0, 2, 3, 1, 4)
    return np.ascontiguousarray(v).astype(BF16)


def _pack_w(w, t=8):
    # [1024, 256] -> [128, 2(ct), 8(ht), 128] partition-major, ct-contiguous
    v = _pack_pm(w, t)                                  # [128, 8, 256]
    v = v.reshape(128, t, 2, 128).transpose(0, 2, 1, 3)
    return np.ascontiguousarray(v).astype(BF16)


def _in_maps(x, y, Wq, Wk, Wv, Wo):
    maps = []
    for core in range(8):
        b, g = core // GP, core % GP
        cs = slice(g * CH, (g + 1) * CH)
        maps.append({
            "xT": _pack_act(x[b]),
            "yT": _pack_act(y[b]),
            "wq": _pack_w(Wq[:, cs] * np.float32(0.125)),
            "wk": _pack_w(Wk[:, cs]),
            "wv": np.ascontiguousarray(_pack_pm(Wv[:, cs], 8)).astype(BF16),
            "wo": np.ascontiguousarray(_pack_pm(Wo[cs, :], 2)).astype(BF16),
        })
    return maps


def _install_ntff_hook():
    """Provide the antenv.axon_hooks shim missing from this container so
    run_bass_kernel_spmd(trace=True) can drive NTFF profiling via ctypes."""
    import sys
    import types
    try:
        from antenv.axon_hooks import get_axon_ntff_profile_hook  # noqa: F401
        return
    except ImportError:
        pass
    from trn_agent_boot.trn_boot import _ntff_profile_via_ctypes
    hook = _ntff_profile_via_ctypes("/opt/axon/libaxon_pjrt.so")
    mod = types.ModuleType("antenv.axon_hooks")
    mod.get_axon_ntff_profile_hook = lambda: hook
    mod.set_axon_ntff_profile_hook = lambda h: None
    sys.modules["antenv.axon_hooks"] = mod


def _run(inputs, trace=False):
    from concourse import bass_utils

    if trace:
        _install_ntff_hook()

    x, y, bias = inputs["x"], inputs["y"], inputs["bias"]
    if np.count_nonzero(np.asarray(bias)):
        raise NotImplementedError("nonzero attention bias not supported")
    nc = _get_nc()
    maps = _in_maps(np.asarray(x, np.float32), np.asarray(y, np.float32),
                    np.asarray(inputs["Wq"], np.float32),
                    np.asarray(inputs["Wk"], np.float32),
                    np.asarray(inputs["Wv"], np.float32),
                    np.asarray(inputs["Wo"], np.float32))
    res = bass_utils.run_bass_kernel_spmd(
        nc, maps, list(range(8)), trace=trace)
    out = np.zeros((B, L, H), np.float32)
    for core in range(8):
        out[core // GP] += res.results[core]["out"].astype(np.float32)
    return out, res


def kernel(**inputs):
    out, _ = _run(inputs, trace=False)
    return out

